# revision 28
# baseline (speedup 1.0000x reference)
"""Trainium2 Bass kernel for nn_BigramEncoder (winner-take-all masked attention).

Math per batch element b (T=2048 tokens, C=512 features, N=1024 nodes):
    q   = Wq @ x_b * C**-0.5 + bq * C**-0.5        [N, C]  (scale folded on host)
    k   = x_b @ Wk.T                               [T, C]
    v   = x_b @ Wv.T                               [T, C]
    sim = q @ k.T                                  [N, T]
    winner-take-all over nodes: for each token t only rows n with
    sim[n,t] == max_n sim[:,t] survive; per-row softmax over the surviving set.
    out = att @ v @ Wout.T + bout                  [N, C]

Key structural facts exploited on-device:
  * Winners satisfy sim[n,t] == amax[t] EXACTLY (amax computed from the same
    values), and softmax is shift/scale invariant, so the att numerator at a
    winner is g[t] = exp(amax[t]).  No dense exp is needed:
        M[t,n]   = (sim[t,n] >= amax[t]) * g[t]     (one DVE tensor_scalar op)
        Z[n]     = sum_t M[t,n]                     (PE ones-matmul)
        att[n,t] = M[t,n] / Z[n]                    (0 rows where Z == 0)
  * The whole kernel runs in a transposed layout (simT [t, n]) so the WTA max
    is a fast free-axis reduction; att is emitted as attT [T, N] and
    transposed on the host, out is emitted natural [N, C].
  * The sim path runs in true fp32 (argmax winners are precision-critical:
    fp32r's ~1.6e-4 error would flip ~14 winners per batch).  k is never
    materialized: sim = q.(x Wk^T)^T is reassociated as (q Wk).x^T, so the
    Wk contraction costs N*C*C MACs instead of T*C*C (half, since N < T).
    The smooth value path (v, att@v, Z, Wout) runs in fp32r (tf32-like) at
    2x the fp32 matmul rate.

Sharding: data-parallel over batch, one batch element per NeuronCore (8 cores).
All weights replicated; no cross-core communication.
"""

import numpy as np

B, T, C, N = 8, 2048, 512, 1024
TB, CB, NB = T // 128, C // 128, N // 128  # 16, 4, 8

_CACHE = {}


def _build_nc():
    from contextlib import ExitStack

    import concourse.bacc as bacc
    import concourse.bass as bass
    import concourse.tile as tile
    from concourse import mybir

    f32 = mybir.dt.float32
    f32r = mybir.dt.float32r
    ALU = mybir.AluOpType
    AFT = mybir.ActivationFunctionType

    nc = bacc.Bacc("TRN2", target_bir_lowering=False, debug=False)

    x_d = nc.dram_tensor("x", [T, C], f32, kind="ExternalInput").ap()
    xT_d = nc.dram_tensor("xT", [C, T], f32, kind="ExternalInput").ap()
    # f32r-declared views for the value path (raw fp32 bits; PE rounds tf32-style)
    xr_d = nc.dram_tensor("xr", [T, C], f32r, kind="ExternalInput").ap()
    wqT_d = nc.dram_tensor("wqT", [T, N], f32, kind="ExternalInput").ap()
    bqb_d = nc.dram_tensor("bqb", [128, N], f32, kind="ExternalInput").ap()
    wk_d = nc.dram_tensor("wk", [C, C], f32, kind="ExternalInput").ap()
    wcT_d = nc.dram_tensor("wcT", [C, C], f32r, kind="ExternalInput").ap()
    botp_d = nc.dram_tensor("botp", [128, CB], f32, kind="ExternalInput").ap()
    onesc_d = nc.dram_tensor("onesc", [128, 1], f32r, kind="ExternalInput").ap()
    ident_d = nc.dram_tensor("ident", [128, 128], f32, kind="ExternalInput").ap()
    attT_d = nc.dram_tensor("attT", [T, N], f32, kind="ExternalOutput").ap()
    out_d = nc.dram_tensor("out", [C, N], f32, kind="ExternalOutput").ap()  # resT; host transposes

    with tile.TileContext(nc) as tc, ExitStack() as top:
        # ---- persistent SBUF (qT, kT, v live across most of the kernel) ----
        pers = top.enter_context(tc.tile_pool(name="pers", bufs=1))
        xT_sb = pers.tile([128, CB * T], f32, tag="xT")        # [c, (jc t)] 32KB/p
        qWT_sb = pers.tile([128, CB * N], f32, tag="qWT")      # [c, (jc n)] 16KB/p
        xn_sb = pers.tile([128, TB * C], f32r, tag="xn")       # x natural [t, (it c)]
        onesc_sb = pers.tile([128, 1], f32r, tag="onesc")
        ident_sb = pers.tile([128, 128], f32, tag="ident")
        ones_row = pers.tile([1, 128], f32, tag="ones_row")
        nc.sync.dma_start(onesc_sb[:], onesc_d)
        nc.sync.dma_start(ident_sb[:], ident_d)
        nc.vector.memset(ones_row[:], 1.0)

        # ================= phase A: q/k/v projections =================
        with tc.tile_pool(name="poolA", bufs=1) as pa, \
             tc.tile_pool(name="wqstream", bufs=3) as pwq:
            qT_sb = pa.tile([128, CB * N], f32, tag="qT")      # [d, (jd n)]
            wk_sb = pa.tile([128, CB * C], f32, tag="wk")      # [d, (jd c)]
            bqb_sb = pa.tile([128, N], f32, tag="bqb")

            # ---- P1: qT[c, n] = sum_t x[t, c] * wqT[t, n]   (fp32) ----
            # Startup latency: P1's i-th step needs only x tile i and wqT tile
            # i, so stream those per-iteration (separate tiles keep the dep
            # granularity); the bulk loads (needed from P2 on) are issued after
            # the loop so they queue behind the streamed tiles.
            with tc.tile_pool(name="psA", bufs=1, space="PSUM") as psA, \
                 tc.tile_pool(name="xstream", bufs=6) as pxs:
                q_ps = [psA.tile([128, 512], f32, tag=f"qps{j}", bufs=1, name=f"qps{j}")
                        for j in range(8)]
                for i in range(TB):
                    x_t = pxs.tile([128, C], f32, tag="xt")
                    nc.sync.dma_start(x_t[:], x_d[i * 128:(i + 1) * 128, :])
                    wq_t = pwq.tile([128, N], f32, tag="wqt")
                    nc.sync.dma_start(wq_t[:], wqT_d[i * 128:(i + 1) * 128, :])
                    for cb in range(CB):
                        lhsT = x_t[:, cb * 128:(cb + 1) * 128]
                        for nh in range(2):
                            nc.tensor.matmul(
                                q_ps[cb * 2 + nh][:],
                                lhsT,
                                wq_t[:, nh * 512:(nh + 1) * 512],
                                start=(i == 0),
                                stop=(i == TB - 1),
                            )
                nc.sync.dma_start(
                    xT_sb[:].rearrange("p (j t) -> p j t", t=T),
                    xT_d.rearrange("(j p) t -> p j t", p=128),
                )
                nc.sync.dma_start(
                    xn_sb[:].rearrange("p (i c) -> p i c", c=C),
                    xr_d.rearrange("(i p) c -> p i c", p=128),
                )
                nc.sync.dma_start(
                    wk_sb[:].rearrange("p (j c) -> p j c", c=C),
                    wk_d.rearrange("(j p) c -> p j c", p=128),
                )
                nc.sync.dma_start(bqb_sb[:], bqb_d)
                for cb in range(CB):
                    for nh in range(2):
                        nc.vector.tensor_add(
                            qT_sb[:, cb * N + nh * 512: cb * N + nh * 512 + 512],
                            q_ps[cb * 2 + nh][:],
                            bqb_sb[:, nh * 512:(nh + 1) * 512],
                        )

            # ---- P2: qWT[c, n] = sum_d wk[d, c] * qT[d, n]   (fp32) ----
            # Reassociation of sim = q.(x Wk^T)^T = (q Wk).x^T: contracting
            # q with Wk first costs N*C*C MACs, half of k's T*C*C.
            with tc.tile_pool(name="psK", bufs=1, space="PSUM") as psK:
                qw_ps = [psK.tile([128, 512], f32, tag=f"qwps{j}", bufs=1,
                                  name=f"qwps{j}") for j in range(8)]
                for cb in range(CB):
                    for jd in range(CB):
                        lhsT = wk_sb[:, jd * C + cb * 128: jd * C + (cb + 1) * 128]
                        for nh in range(2):
                            nc.tensor.matmul(
                                qw_ps[cb * 2 + nh][:],
                                lhsT,
                                qT_sb[:, jd * N + nh * 512: jd * N + nh * 512 + 512],
                                start=(jd == 0),
                                stop=(jd == CB - 1),
                            )
                    for nh in range(2):
                        nc.scalar.copy(
                            qWT_sb[:, cb * N + nh * 512: cb * N + nh * 512 + 512],
                            qw_ps[cb * 2 + nh][:],
                        )

        # ================= phase B: masked attention =================
        with tc.tile_pool(name="poolB", bufs=1) as pb, \
             tc.tile_pool(name="stats", bufs=4) as pstat, \
             tc.tile_pool(name="attrot", bufs=3) as patt, \
             tc.tile_pool(name="resrot", bufs=2) as pres:
            M_sb = pb.tile([128, TB * N], f32r, tag="M")       # [t, (it n)] 64KB/p
            wcT_sb = pb.tile([128, CB * C], f32r, tag="wcT")   # (Wout Wv)^T [e, (je d)]
            botp_sb = pb.tile([128, CB], f32, tag="botp")      # bout as [128, db]
            xM_sb = pb.tile([128, CB * N], f32r, tag="xM")     # [e, (je n)]
            Rb_sb = pb.tile([128, N], f32, tag="Rb")
            Rp_sb = pb.tile([128, 8], f32, tag="Rp")
            z_sb = pb.tile([1, N], f32, tag="z")
            r_sb = pb.tile([1, N], f32, tag="r")

            nc.sync.dma_start(
                wcT_sb[:].rearrange("p (j d) -> p j d", d=C),
                wcT_d.rearrange("(j p) d -> p j d", p=128),
            )
            nc.sync.dma_start(botp_sb[:], botp_d)

            # ---- P4: simT[t, n] = sum_c xT[c, t] * qWT[c, n] (fp32); WTA mask ----
            with tc.tile_pool(name="psB", bufs=2, space="PSUM") as psB, \
                 tc.tile_pool(name="psZ", bufs=1, space="PSUM") as psZ:
                z_ps = [psZ.tile([1, 512], f32, tag=f"z{nh}", bufs=1, name=f"zps{nh}") for nh in range(2)]
                for i in range(TB):
                    sim_ps = psB.tile([128, N], f32, tag="sim")
                    for jf in range(CB):
                        lhsT = xT_sb[:, jf * T + i * 128: jf * T + i * 128 + 128]
                        for nh in range(2):
                            nc.tensor.matmul(
                                sim_ps[:, nh * 512: nh * 512 + 512],
                                lhsT,
                                qWT_sb[:, jf * N + nh * 512: jf * N + nh * 512 + 512],
                                start=(jf == 0),
                                stop=(jf == CB - 1),
                            )
                    amax = pstat.tile([128, 1], f32, tag="amax")
                    g = pstat.tile([128, 1], f32, tag="g")
                    nc.vector.reduce_max(amax[:], sim_ps[:], axis=mybir.AxisListType.X)
                    nc.scalar.activation(g[:], amax[:], AFT.Exp)
                    Mv = M_sb[:, i * N:(i + 1) * N]
                    nc.vector.tensor_scalar(
                        Mv, sim_ps[:], amax[:], g[:], op0=ALU.is_ge, op1=ALU.mult
                    )
                    for nh in range(2):
                        nc.tensor.matmul(
                            z_ps[nh][:],
                            onesc_sb[:],
                            M_sb[:, i * N + nh * 512: i * N + nh * 512 + 512],
                            start=(i == 0),
                            stop=(i == TB - 1),
                        )

                # ---- normalizers, both layouts, off the PE critical path:
                #   Rp    [128, 8] -> per-partition scalars (applied at P7 evac)
                #   R_row [1, N]   -> Rb broadcast (for the att output)
                # The reciprocal runs in the [128, 8] layout (8 elems/lane,
                # ~0.1us); a [1, N] single-lane reciprocal costs ~6.5us. The
                # layout round-trip is plain K=1 / identity matmuls on the PE.
                for nh in range(2):
                    nc.vector.tensor_copy(z_sb[:, nh * 512: nh * 512 + 512], z_ps[nh][:])
                nc.vector.tensor_scalar_max(r_sb[:], z_sb[:], 1e-30)
                zp_ps = psZ.tile([128, 8], f32, tag="zp", bufs=1)
                for nb in range(NB):
                    nc.tensor.matmul(
                        zp_ps[:, nb:nb + 1],
                        r_sb[:, nb * 128:(nb + 1) * 128],
                        ones_row[:, 0:1],
                        start=True,
                        stop=True,
                    )
                nc.vector.reciprocal(Rp_sb[:], zp_ps[:])
                for nh in range(2):
                    for j in range(4):
                        nb = nh * 4 + j
                        nc.tensor.matmul(
                            z_ps[nh][:, j * 128:(j + 1) * 128],
                            Rp_sb[:, nb:nb + 1],
                            ident_sb[:],
                            start=True,
                            stop=True,
                        )
                for nh in range(2):
                    nc.vector.tensor_copy(r_sb[:, nh * 512: nh * 512 + 512], z_ps[nh][:])
                for nh in range(2):
                    rb_ps = psZ.tile([128, 512], f32, tag="rb", bufs=1)
                    nc.tensor.matmul(
                        rb_ps[:],
                        ones_row[:],
                        r_sb[:, nh * 512: nh * 512 + 512],
                        start=True,
                        stop=True,
                    )
                    nc.scalar.copy(Rb_sb[:, nh * 512: nh * 512 + 512], rb_ps[:])

            # ---- P6a: xM[e, n] = sum_t x[t, e] * M[t, n]  (unnormalized, so
            # the PE never waits on the Z/R chain); attT = M * R feeds only the
            # att DMA and trails slightly behind.  v is never materialized:
            # out = att.(x Wv^T).Wout^T is reassociated as (Wout Wv).(x^T M),
            # with Wc = Wout Wv computed on the host (weights only).
            # Two 4-bank pools + two cb passes: the first half of the xM
            # accumulators starts on the sim pool's early-released banks
            # instead of waiting for the Z/R chain to free the second zone.
            with tc.tile_pool(name="psC1", bufs=1, space="PSUM") as psC1, \
                 tc.tile_pool(name="psC2", bufs=1, space="PSUM") as psC2:
                o_ps = [
                    (psC1 if j < 4 else psC2).tile(
                        [128, 512], f32, tag=f"ops{j}", bufs=1, name=f"ops{j}"
                    )
                    for j in range(8)
                ]
                for i in range(TB):
                    for cb in range(2):
                        lhsT = xn_sb[:, i * C + cb * 128: i * C + (cb + 1) * 128]
                        for nh in range(2):
                            nc.tensor.matmul(
                                o_ps[cb * 2 + nh][:],
                                lhsT,
                                M_sb[:, i * N + nh * 512: i * N + nh * 512 + 512],
                                start=(i == 0),
                                stop=(i == TB - 1),
                            )
                    attT_t = patt.tile([128, N], f32, tag="attT")
                    nc.vector.tensor_mul(
                        attT_t[:],
                        M_sb[:, i * N:(i + 1) * N].bitcast(f32),
                        Rb_sb[:],
                    )
                    nc.sync.dma_start(attT_d[i * 128:(i + 1) * 128, :], attT_t[:])
                for i in range(TB):
                    for cb in range(2, CB):
                        lhsT = xn_sb[:, i * C + cb * 128: i * C + (cb + 1) * 128]
                        for nh in range(2):
                            nc.tensor.matmul(
                                o_ps[cb * 2 + nh][:],
                                lhsT,
                                M_sb[:, i * N + nh * 512: i * N + nh * 512 + 512],
                                start=(i == 0),
                                stop=(i == TB - 1),
                            )
                for cb in range(CB):
                    for nh in range(2):
                        dst = xM_sb[:, cb * N + nh * 512: cb * N + nh * 512 + 512]
                        # split the 8 evacuations across DVE and ACT to halve
                        # the barrier before P6b
                        if (cb * 2 + nh) % 2 == 0:
                            nc.vector.tensor_copy(dst, o_ps[cb * 2 + nh][:])
                        else:
                            nc.scalar.copy(dst, o_ps[cb * 2 + nh][:])

            # ---- P6b: resT[d, n] = R[n] * sum_e wcT[e, d] * xM[e, n] + bout[d] ----
            with tc.tile_pool(name="psD", bufs=1, space="PSUM") as psD:
                r_ps = [psD.tile([128, 512], f32, tag=f"rps{j}", bufs=1, name=f"rps{j}") for j in range(8)]
                for db in range(CB):
                    for je in range(CB):
                        lhsT = wcT_sb[:, je * C + db * 128: je * C + (db + 1) * 128]
                        for nh in range(2):
                            nc.tensor.matmul(
                                r_ps[db * 2 + nh][:],
                                lhsT,
                                xM_sb[:, je * N + nh * 512: je * N + nh * 512 + 512],
                                start=(je == 0),
                                stop=(je == CB - 1),
                            )
                    for nh in range(2):
                        res_t = pres.tile([128, 512], f32, tag="res_t")
                        nc.vector.tensor_mul(
                            res_t[:], r_ps[db * 2 + nh][:],
                            Rb_sb[:, nh * 512:(nh + 1) * 512],
                        )
                        res_b = pres.tile([128, 512], f32, tag="res_b")
                        nc.scalar.activation(
                            res_b[:], res_t[:], AFT.Identity,
                            bias=botp_sb[:, db:db + 1],
                        )
                        nc.sync.dma_start(
                            out_d[db * 128:(db + 1) * 128, nh * 512:(nh + 1) * 512],
                            res_b[:],
                        )

    nc.compile()
    return nc


def _get_nc():
    if "nc" not in _CACHE:
        _CACHE["nc"] = _build_nc()
    return _CACHE["nc"]


def _make_in_maps(inputs):
    x = np.ascontiguousarray(np.asarray(inputs["x"], dtype=np.float32))
    Wq = np.asarray(inputs["Wq"], dtype=np.float32)
    bq = np.asarray(inputs["bq"], dtype=np.float32)
    Wk = np.asarray(inputs["Wk"], dtype=np.float32)
    Wv = np.asarray(inputs["Wv"], dtype=np.float32)
    Wout = np.asarray(inputs["Wout"], dtype=np.float32)
    bout = np.asarray(inputs["bout"], dtype=np.float32)

    scale = np.float32(C ** -0.5)
    shared = {
        "wqT": np.ascontiguousarray((Wq * scale).T),
        "bqb": np.ascontiguousarray(np.tile((bq * scale)[None, :], (128, 1))),
        "wk": np.ascontiguousarray(Wk),
        "wcT": np.ascontiguousarray((Wout @ Wv).T),
        "botp": np.ascontiguousarray(bout.reshape(CB, 128).T),
        "onesc": np.ones((128, 1), dtype=np.float32),
        "ident": np.eye(128, dtype=np.float32),
    }
    in_maps = []
    for b in range(B):
        m = dict(shared)
        m["x"] = np.ascontiguousarray(x[b])
        m["xT"] = np.ascontiguousarray(x[b].T)
        m["xr"] = m["x"]
        in_maps.append(m)
    return in_maps


def run(inputs, **spmd_kwargs):
    """Build/compile (cached), run on 8 cores, return (out, att) plus raw results."""
    from concourse import bass_utils

    nc = _get_nc()
    in_maps = _make_in_maps(inputs)
    res = bass_utils.run_bass_kernel_spmd(nc, in_maps, core_ids=list(range(B)), **spmd_kwargs)
    out = np.stack(
        [np.ascontiguousarray(res.results[b]["out"].T) for b in range(B)]
    ).astype(np.float32)
    att = np.stack(
        [np.ascontiguousarray(res.results[b]["attT"].T) for b in range(B)]
    ).astype(np.float32)
    return (out, att), res


def kernel(**inputs):
    (out, att), _ = run(inputs)
    return out, att


# revision 29
# speedup vs baseline: 1.0171x; 1.0171x over previous
"""Trainium2 Bass kernel for nn_BigramEncoder (winner-take-all masked attention).

Math per batch element b (T=2048 tokens, C=512 features, N=1024 nodes):
    q   = Wq @ x_b * C**-0.5 + bq * C**-0.5        [N, C]  (scale folded on host)
    k   = x_b @ Wk.T                               [T, C]
    v   = x_b @ Wv.T                               [T, C]
    sim = q @ k.T                                  [N, T]
    winner-take-all over nodes: for each token t only rows n with
    sim[n,t] == max_n sim[:,t] survive; per-row softmax over the surviving set.
    out = att @ v @ Wout.T + bout                  [N, C]

Key structural facts exploited on-device:
  * Winners satisfy sim[n,t] == amax[t] EXACTLY (amax computed from the same
    values), and softmax is shift/scale invariant, so the att numerator at a
    winner is g[t] = exp(amax[t]).  No dense exp is needed:
        M[t,n]   = (sim[t,n] >= amax[t]) * g[t]     (one DVE tensor_scalar op)
        Z[n]     = sum_t M[t,n]                     (PE ones-matmul)
        att[n,t] = M[t,n] / Z[n]                    (0 rows where Z == 0)
  * The whole kernel runs in a transposed layout (simT [t, n]) so the WTA max
    is a fast free-axis reduction; att is emitted as attT [T, N] and
    transposed on the host, out is emitted natural [N, C].
  * The sim path runs in true fp32 (argmax winners are precision-critical:
    fp32r's ~1.6e-4 error would flip ~14 winners per batch).  k is never
    materialized: sim = q.(x Wk^T)^T is reassociated as (q Wk).x^T, so the
    Wk contraction costs N*C*C MACs instead of T*C*C (half, since N < T).
    The smooth value path (v, att@v, Z, Wout) runs in fp32r (tf32-like) at
    2x the fp32 matmul rate.

Sharding: data-parallel over batch, one batch element per NeuronCore (8 cores).
All weights replicated; no cross-core communication.
"""

import numpy as np

B, T, C, N = 8, 2048, 512, 1024
TB, CB, NB = T // 128, C // 128, N // 128  # 16, 4, 8

_CACHE = {}


def _build_nc():
    from contextlib import ExitStack

    import concourse.bacc as bacc
    import concourse.bass as bass
    import concourse.tile as tile
    from concourse import mybir

    f32 = mybir.dt.float32
    f32r = mybir.dt.float32r
    ALU = mybir.AluOpType
    AFT = mybir.ActivationFunctionType

    nc = bacc.Bacc("TRN2", target_bir_lowering=False, debug=False)

    x_d = nc.dram_tensor("x", [T, C], f32, kind="ExternalInput").ap()
    xT_d = nc.dram_tensor("xT", [C, T], f32, kind="ExternalInput").ap()
    # f32r-declared views for the value path (raw fp32 bits; PE rounds tf32-style)
    xr_d = nc.dram_tensor("xr", [T, C], f32r, kind="ExternalInput").ap()
    wqT_d = nc.dram_tensor("wqT", [T, N], f32, kind="ExternalInput").ap()
    bqb_d = nc.dram_tensor("bqb", [128, N], f32, kind="ExternalInput").ap()
    wk_d = nc.dram_tensor("wk", [C, C], f32, kind="ExternalInput").ap()
    wcT_d = nc.dram_tensor("wcT", [C, C], f32r, kind="ExternalInput").ap()
    botp_d = nc.dram_tensor("botp", [128, CB], f32, kind="ExternalInput").ap()
    onesc_d = nc.dram_tensor("onesc", [128, 1], f32r, kind="ExternalInput").ap()
    ident_d = nc.dram_tensor("ident", [128, 128], f32, kind="ExternalInput").ap()
    attT_d = nc.dram_tensor("attT", [T, N], f32, kind="ExternalOutput").ap()
    out_d = nc.dram_tensor("out", [C, N], f32, kind="ExternalOutput").ap()  # resT; host transposes

    with tile.TileContext(nc) as tc, ExitStack() as top:
        # ---- persistent SBUF (qT, kT, v live across most of the kernel) ----
        pers = top.enter_context(tc.tile_pool(name="pers", bufs=1))
        xT_sb = pers.tile([128, CB * T], f32, tag="xT")        # [c, (jc t)] 32KB/p
        qWT_sb = pers.tile([128, CB * N], f32, tag="qWT")      # [c, (jc n)] 16KB/p
        xn_sb = pers.tile([128, TB * C], f32r, tag="xn")       # x natural [t, (it c)]
        onesc_sb = pers.tile([128, 1], f32r, tag="onesc")
        ident_sb = pers.tile([128, 128], f32, tag="ident")
        ones_row = pers.tile([1, 128], f32, tag="ones_row")
        nc.sync.dma_start(onesc_sb[:], onesc_d)
        nc.sync.dma_start(ident_sb[:], ident_d)
        nc.vector.memset(ones_row[:], 1.0)

        # ================= phase A: q/k/v projections =================
        with tc.tile_pool(name="poolA", bufs=1) as pa, \
             tc.tile_pool(name="wqstream", bufs=3) as pwq:
            qT_sb = pa.tile([128, CB * N], f32, tag="qT")      # [d, (jd n)]
            wk_sb = pa.tile([128, CB * C], f32, tag="wk")      # [d, (jd c)]
            bqb_sb = pa.tile([128, N], f32, tag="bqb")

            # ---- P1: qT[c, n] = sum_t x[t, c] * wqT[t, n]   (fp32) ----
            # Startup latency: P1's i-th step needs only x tile i and wqT tile
            # i, so stream those per-iteration (separate tiles keep the dep
            # granularity); the bulk loads (needed from P2 on) are issued after
            # the loop so they queue behind the streamed tiles.
            with tc.tile_pool(name="psA", bufs=1, space="PSUM") as psA, \
                 tc.tile_pool(name="xstream", bufs=6) as pxs:
                q_ps = [psA.tile([128, 512], f32, tag=f"qps{j}", bufs=1, name=f"qps{j}")
                        for j in range(8)]
                for i in range(TB):
                    x_t = pxs.tile([128, C], f32, tag="xt")
                    nc.sync.dma_start(x_t[:], x_d[i * 128:(i + 1) * 128, :])
                    wq_t = pwq.tile([128, N], f32, tag="wqt")
                    nc.sync.dma_start(wq_t[:], wqT_d[i * 128:(i + 1) * 128, :])
                    for cb in range(CB):
                        lhsT = x_t[:, cb * 128:(cb + 1) * 128]
                        for nh in range(2):
                            nc.tensor.matmul(
                                q_ps[cb * 2 + nh][:],
                                lhsT,
                                wq_t[:, nh * 512:(nh + 1) * 512],
                                start=(i == 0),
                                stop=(i == TB - 1),
                            )
                nc.sync.dma_start(
                    xT_sb[:].rearrange("p (j t) -> p j t", t=T),
                    xT_d.rearrange("(j p) t -> p j t", p=128),
                )
                nc.sync.dma_start(
                    xn_sb[:].rearrange("p (i c) -> p i c", c=C),
                    xr_d.rearrange("(i p) c -> p i c", p=128),
                )
                nc.sync.dma_start(
                    wk_sb[:].rearrange("p (j c) -> p j c", c=C),
                    wk_d.rearrange("(j p) c -> p j c", p=128),
                )
                nc.sync.dma_start(bqb_sb[:], bqb_d)
                for cb in range(CB):
                    for nh in range(2):
                        nc.vector.tensor_add(
                            qT_sb[:, cb * N + nh * 512: cb * N + nh * 512 + 512],
                            q_ps[cb * 2 + nh][:],
                            bqb_sb[:, nh * 512:(nh + 1) * 512],
                        )

            # ---- P2: qWT[c, n] = sum_d wk[d, c] * qT[d, n]   (fp32) ----
            # Reassociation of sim = q.(x Wk^T)^T = (q Wk).x^T: contracting
            # q with Wk first costs N*C*C MACs, half of k's T*C*C.
            with tc.tile_pool(name="psK", bufs=1, space="PSUM") as psK:
                qw_ps = [psK.tile([128, 512], f32, tag=f"qwps{j}", bufs=1,
                                  name=f"qwps{j}") for j in range(8)]
                for cb in range(CB):
                    for jd in range(CB):
                        lhsT = wk_sb[:, jd * C + cb * 128: jd * C + (cb + 1) * 128]
                        for nh in range(2):
                            nc.tensor.matmul(
                                qw_ps[cb * 2 + nh][:],
                                lhsT,
                                qT_sb[:, jd * N + nh * 512: jd * N + nh * 512 + 512],
                                start=(jd == 0),
                                stop=(jd == CB - 1),
                            )
                    for nh in range(2):
                        nc.scalar.copy(
                            qWT_sb[:, cb * N + nh * 512: cb * N + nh * 512 + 512],
                            qw_ps[cb * 2 + nh][:],
                        )

        # ================= phase B: masked attention =================
        with tc.tile_pool(name="poolB", bufs=1) as pb, \
             tc.tile_pool(name="stats", bufs=4) as pstat, \
             tc.tile_pool(name="attrot", bufs=3) as patt, \
             tc.tile_pool(name="resrot", bufs=2) as pres:
            M_sb = pb.tile([128, TB * N], f32r, tag="M")       # [t, (it n)] 64KB/p
            wcT_sb = pb.tile([128, CB * C], f32r, tag="wcT")   # (Wout Wv)^T [e, (je d)]
            botp_sb = pb.tile([128, CB], f32, tag="botp")      # bout as [128, db]
            xM_sb = pb.tile([128, CB * N], f32r, tag="xM")     # [e, (je n)]
            Rb_sb = pb.tile([128, N], f32, tag="Rb")
            Rp_sb = pb.tile([128, 8], f32, tag="Rp")
            z_sb = pb.tile([1, N], f32, tag="z")
            r_sb = pb.tile([1, N], f32, tag="r")

            nc.sync.dma_start(
                wcT_sb[:].rearrange("p (j d) -> p j d", d=C),
                wcT_d.rearrange("(j p) d -> p j d", p=128),
            )
            nc.sync.dma_start(botp_sb[:], botp_d)

            # ---- P4: simT[t, n] = sum_c xT[c, t] * qWT[c, n] (fp32); WTA mask ----
            with tc.tile_pool(name="psB", bufs=2, space="PSUM") as psB, \
                 tc.tile_pool(name="psZ", bufs=1, space="PSUM") as psZ:
                z_ps = [psZ.tile([1, 512], f32, tag=f"z{nh}", bufs=1, name=f"zps{nh}") for nh in range(2)]
                for i in range(TB):
                    sim_ps = psB.tile([128, N], f32, tag="sim")
                    for jf in range(CB):
                        lhsT = xT_sb[:, jf * T + i * 128: jf * T + i * 128 + 128]
                        for nh in range(2):
                            nc.tensor.matmul(
                                sim_ps[:, nh * 512: nh * 512 + 512],
                                lhsT,
                                qWT_sb[:, jf * N + nh * 512: jf * N + nh * 512 + 512],
                                start=(jf == 0),
                                stop=(jf == CB - 1),
                            )
                    amax = pstat.tile([128, 1], f32, tag="amax")
                    g = pstat.tile([128, 1], f32, tag="g")
                    nc.vector.reduce_max(amax[:], sim_ps[:], axis=mybir.AxisListType.X)
                    nc.scalar.activation(g[:], amax[:], AFT.Exp)
                    Mv = M_sb[:, i * N:(i + 1) * N]
                    nc.vector.tensor_scalar(
                        Mv, sim_ps[:], amax[:], g[:], op0=ALU.is_ge, op1=ALU.mult
                    )
                    for nh in range(2):
                        nc.tensor.matmul(
                            z_ps[nh][:],
                            onesc_sb[:],
                            M_sb[:, i * N + nh * 512: i * N + nh * 512 + 512],
                            start=(i == 0),
                            stop=(i == TB - 1),
                        )

                # ---- normalizers, both layouts, off the PE critical path:
                #   Rp    [128, 8] -> per-partition scalars (applied at P7 evac)
                #   R_row [1, N]   -> Rb broadcast (for the att output)
                # The reciprocal runs in the [128, 8] layout (8 elems/lane,
                # ~0.1us); a [1, N] single-lane reciprocal costs ~6.5us. The
                # layout round-trip is plain K=1 / identity matmuls on the PE.
                for nh in range(2):
                    nc.vector.tensor_copy(z_sb[:, nh * 512: nh * 512 + 512], z_ps[nh][:])
                nc.vector.tensor_scalar_max(r_sb[:], z_sb[:], 1e-30)
                zp_ps = psZ.tile([128, 8], f32, tag="zp", bufs=1)
                for nb in range(NB):
                    nc.tensor.matmul(
                        zp_ps[:, nb:nb + 1],
                        r_sb[:, nb * 128:(nb + 1) * 128],
                        ones_row[:, 0:1],
                        start=True,
                        stop=True,
                    )
                nc.vector.reciprocal(Rp_sb[:], zp_ps[:])
                for nh in range(2):
                    for j in range(4):
                        nb = nh * 4 + j
                        nc.tensor.matmul(
                            z_ps[nh][:, j * 128:(j + 1) * 128],
                            Rp_sb[:, nb:nb + 1],
                            ident_sb[:],
                            start=True,
                            stop=True,
                        )
                for nh in range(2):
                    nc.vector.tensor_copy(r_sb[:, nh * 512: nh * 512 + 512], z_ps[nh][:])
                for nh in range(2):
                    rb_ps = psZ.tile([128, 512], f32, tag="rb", bufs=1)
                    nc.tensor.matmul(
                        rb_ps[:],
                        ones_row[:],
                        r_sb[:, nh * 512: nh * 512 + 512],
                        start=True,
                        stop=True,
                    )
                    nc.scalar.copy(Rb_sb[:, nh * 512: nh * 512 + 512], rb_ps[:])

            # ---- P6a: xM[e, n] = sum_t x[t, e] * M[t, n]  (unnormalized, so
            # the PE never waits on the Z/R chain); attT = M * R feeds only the
            # att DMA and trails slightly behind.  v is never materialized:
            # out = att.(x Wv^T).Wout^T is reassociated as (Wout Wv).(x^T M),
            # with Wc = Wout Wv computed on the host (weights only).
            with tc.tile_pool(name="psC", bufs=1, space="PSUM") as psC:
                o_ps = [psC.tile([128, 512], f32, tag=f"ops{j}", bufs=1, name=f"ops{j}") for j in range(8)]
                for i in range(TB):
                    for cb in range(CB):
                        lhsT = xn_sb[:, i * C + cb * 128: i * C + (cb + 1) * 128]
                        for nh in range(2):
                            nc.tensor.matmul(
                                o_ps[cb * 2 + nh][:],
                                lhsT,
                                M_sb[:, i * N + nh * 512: i * N + nh * 512 + 512],
                                start=(i == 0),
                                stop=(i == TB - 1),
                            )
                    attT_t = patt.tile([128, N], f32, tag="attT")
                    nc.vector.tensor_mul(
                        attT_t[:],
                        M_sb[:, i * N:(i + 1) * N].bitcast(f32),
                        Rb_sb[:],
                    )
                    nc.sync.dma_start(attT_d[i * 128:(i + 1) * 128, :], attT_t[:])
                for cb in range(CB):
                    for nh in range(2):
                        dst = xM_sb[:, cb * N + nh * 512: cb * N + nh * 512 + 512]
                        # split the 8 evacuations across DVE and ACT to halve
                        # the barrier before P6b
                        if (cb * 2 + nh) % 2 == 0:
                            nc.vector.tensor_copy(dst, o_ps[cb * 2 + nh][:])
                        else:
                            nc.scalar.copy(dst, o_ps[cb * 2 + nh][:])

            # ---- P6b: resT[d, n] = R[n] * sum_e wcT[e, d] * xM[e, n] + bout[d] ----
            with tc.tile_pool(name="psD", bufs=1, space="PSUM") as psD:
                r_ps = [psD.tile([128, 512], f32, tag=f"rps{j}", bufs=1, name=f"rps{j}") for j in range(8)]
                for db in range(CB):
                    for je in range(CB):
                        lhsT = wcT_sb[:, je * C + db * 128: je * C + (db + 1) * 128]
                        for nh in range(2):
                            nc.tensor.matmul(
                                r_ps[db * 2 + nh][:],
                                lhsT,
                                xM_sb[:, je * N + nh * 512: je * N + nh * 512 + 512],
                                start=(je == 0),
                                stop=(je == CB - 1),
                            )
                    for nh in range(2):
                        res_t = pres.tile([128, 512], f32, tag="res_t")
                        nc.vector.tensor_mul(
                            res_t[:], r_ps[db * 2 + nh][:],
                            Rb_sb[:, nh * 512:(nh + 1) * 512],
                        )
                        res_b = pres.tile([128, 512], f32, tag="res_b")
                        nc.scalar.activation(
                            res_b[:], res_t[:], AFT.Identity,
                            bias=botp_sb[:, db:db + 1],
                        )
                        nc.sync.dma_start(
                            out_d[db * 128:(db + 1) * 128, nh * 512:(nh + 1) * 512],
                            res_b[:],
                        )

    nc.compile()
    return nc


def _get_nc():
    if "nc" not in _CACHE:
        _CACHE["nc"] = _build_nc()
    return _CACHE["nc"]


def _make_in_maps(inputs):
    x = np.ascontiguousarray(np.asarray(inputs["x"], dtype=np.float32))
    Wq = np.asarray(inputs["Wq"], dtype=np.float32)
    bq = np.asarray(inputs["bq"], dtype=np.float32)
    Wk = np.asarray(inputs["Wk"], dtype=np.float32)
    Wv = np.asarray(inputs["Wv"], dtype=np.float32)
    Wout = np.asarray(inputs["Wout"], dtype=np.float32)
    bout = np.asarray(inputs["bout"], dtype=np.float32)

    scale = np.float32(C ** -0.5)
    shared = {
        "wqT": np.ascontiguousarray((Wq * scale).T),
        "bqb": np.ascontiguousarray(np.tile((bq * scale)[None, :], (128, 1))),
        "wk": np.ascontiguousarray(Wk),
        "wcT": np.ascontiguousarray((Wout @ Wv).T),
        "botp": np.ascontiguousarray(bout.reshape(CB, 128).T),
        "onesc": np.ones((128, 1), dtype=np.float32),
        "ident": np.eye(128, dtype=np.float32),
    }
    in_maps = []
    for b in range(B):
        m = dict(shared)
        m["x"] = np.ascontiguousarray(x[b])
        m["xT"] = np.ascontiguousarray(x[b].T)
        m["xr"] = m["x"]
        in_maps.append(m)
    return in_maps


def run(inputs, **spmd_kwargs):
    """Build/compile (cached), run on 8 cores, return (out, att) plus raw results."""
    from concourse import bass_utils

    nc = _get_nc()
    in_maps = _make_in_maps(inputs)
    res = bass_utils.run_bass_kernel_spmd(nc, in_maps, core_ids=list(range(B)), **spmd_kwargs)
    out = np.stack(
        [np.ascontiguousarray(res.results[b]["out"].T) for b in range(B)]
    ).astype(np.float32)
    att = np.stack(
        [np.ascontiguousarray(res.results[b]["attT"].T) for b in range(B)]
    ).astype(np.float32)
    return (out, att), res


def kernel(**inputs):
    (out, att), _ = run(inputs)
    return out, att
